# revision 9
# baseline (speedup 1.0000x reference)
"""Multi-head attention Trainium2 Bass kernel.

Full problem: B=2, S=2048, HIDDEN=1024, 16 heads x d_head 64.
Sharding over 8 cores: batch (2) x head-group (4 groups of 4 heads).
Each core computes, for its batch b and heads [4g, 4g+4):
  - attn_part [4, S, S]  (normalized attention probabilities)
  - out_part  [S, HIDDEN] = concat(local heads) @ Wo[rows of local heads]
Host gathers: attn[b, 4g:4g+4] = attn_part;  out[b] = sum_g out_part.
"""

import numpy as np
from contextlib import ExitStack

import concourse.bass as bass
import concourse.tile as tile
from concourse import bacc, mybir
from concourse.bass_utils import run_bass_kernel_spmd
from concourse.masks import make_identity

P = 128
DH = 64                      # head dim
F32 = mybir.dt.float32
F32R = mybir.dt.float32r
AF = mybir.ActivationFunctionType
AX = mybir.AxisListType
ALU = mybir.AluOpType


def emit(tc, outs, ins, S, D, HL):
    """Emit one core's program.

    ins:  xq, xk, xv [S, D];  wq, wk, wv [D, HL*DH] (w[d, h*DH+k] = W[h,d,k]);
          wo [HL*DH, D]
    outs: attn [HL, S, S];  out [S, D]
    """
    nc = tc.nc
    assert S % 512 == 0 and D % P == 0 and HL % 2 == 0
    NCH = D // P            # d-chunks
    PAIRS = HL // 2         # head pairs (2 heads stacked on 128 partitions)
    NST = S // P            # s-tiles
    TB = min(512, S)        # t-block for scores/exp/normalize
    NTB = S // TB
    SB = min(256, S)        # s-block for PT strips / AV
    NSB = S // SB
    NTC = S // P            # t-chunks
    HD = HL * DH            # local concat width

    xq_d, xk_d, xv_d = ins["xq"], ins["xk"], ins["xv"]
    wq_d, wk_d, wv_d, wo_d = ins["wq"], ins["wk"], ins["wv"], ins["wo"]
    attn_d, out_d = outs["attn"], outs["out"]

    with ExitStack() as octx:
        const = octx.enter_context(tc.tile_pool(name="const", bufs=1))
        persist = octx.enter_context(tc.tile_pool(name="persist", bufs=1))

        ident_f = const.tile([P, P], F32)
        make_identity(nc, ident_f)
        ident_r = const.tile([P, P], F32R)
        nc.scalar.copy(ident_r[:], ident_f[:])

        # output-projection weights, rounded to f32r: [128, PAIRS, D]
        wo_f = const.tile([P, PAIRS * D], F32)
        nc.sync.dma_start(wo_f[:].rearrange("p (c n) -> p c n", c=PAIRS),
                          wo_d.rearrange("(c p) n -> p c n", p=P))
        wo_r = const.tile([P, PAIRS * D], F32R)
        nc.scalar.copy(wo_r[:], wo_f[:])

        # persistent activations (f32r; feed fp32r matmuls)
        qT = persist.tile([P, PAIRS * S], F32R)   # [2x64 dh, pair-major s]
        kT = persist.tile([P, PAIRS * S], F32R)
        V4 = persist.tile([P, NST * HD], F32R)    # [t, t-tile-major (h, k)]
        OT2 = persist.tile([P, PAIRS * S], F32R)  # [2x64 dh, pair-major s]

        # ---------------- Phase 1: transposes + projections ----------------
        with ExitStack() as ctx:
            xpool = ctx.enter_context(tc.tile_pool(name="xpool", bufs=8))
            xtpool = ctx.enter_context(tc.tile_pool(name="xtpool", bufs=2))
            wfpool = ctx.enter_context(tc.tile_pool(name="wfpool", bufs=2))
            wrpool = ctx.enter_context(tc.tile_pool(name="wrpool", bufs=3))
            ps_xt = ctx.enter_context(tc.tile_pool(name="ps_xt", bufs=3, space="PSUM"))
            ps_pj = ctx.enter_context(tc.tile_pool(name="ps_pj", bufs=3, space="PSUM"))

            for x_d, w_d, kind in ((xk_d, wk_d, "k"), (xq_d, wq_d, "q"),
                                   (xv_d, wv_d, "v")):
                # round projection weights to f32r: [128, NCH, HL*DH]
                w_f = wfpool.tile([P, NCH * HD], F32, tag="w_f")
                nc.sync.dma_start(w_f[:].rearrange("p (c n) -> p c n", c=NCH),
                                  w_d.rearrange("(c p) n -> p c n", p=P))
                w_r = wrpool.tile([P, NCH * HD], F32R, tag="w_r")
                nc.scalar.copy(w_r[:], w_f[:])

                for sblk in range(S // 512):
                    # x natural tiles for this 512-row slab
                    xts = []
                    for st2 in range(4):
                        x_t = xpool.tile([P, D], F32, tag="x_t")
                        nc.sync.dma_start(
                            x_t[:], x_d[sblk * 512 + st2 * P: sblk * 512 + (st2 + 1) * P, :])
                        xts.append(x_t)
                    # transpose slab -> xT [128, NCH*512] (f32r)
                    xT = xtpool.tile([P, NCH * 512], F32R, tag="xT")
                    for c in range(NCH):
                        pxt = ps_xt.tile([P, 512], F32, tag="pxt")
                        for st2 in range(4):
                            nc.tensor.transpose(
                                pxt[:, st2 * P:(st2 + 1) * P],
                                xts[st2][:, c * P:(c + 1) * P], ident_f[:])
                        nc.scalar.copy(xT[:, c * 512:(c + 1) * 512], pxt[:])

                    if kind in ("q", "k"):
                        dst = qT if kind == "q" else kT
                        for pr in range(PAIRS):
                            pp = ps_pj.tile([P, 512], F32, tag="pp")
                            for c in range(NCH):
                                nc.tensor.matmul(
                                    pp[:],
                                    w_r[:, c * HD + pr * P: c * HD + (pr + 1) * P],
                                    xT[:, c * 512:(c + 1) * 512],
                                    start=(c == 0), stop=(c == NCH - 1))
                            nc.scalar.copy(
                                dst[:, pr * S + sblk * 512: pr * S + (sblk + 1) * 512],
                                pp[:])
                    else:
                        for tt2 in range(4):
                            tt = sblk * 4 + tt2
                            pv = ps_pj.tile([P, HD], F32, tag="pp")
                            for c in range(NCH):
                                nc.tensor.matmul(
                                    pv[:],
                                    xT[:, c * 512 + tt2 * P: c * 512 + (tt2 + 1) * P],
                                    w_r[:, c * HD:(c + 1) * HD],
                                    start=(c == 0), stop=(c == NCH - 1))
                            nc.vector.tensor_copy(
                                V4[:, tt * HD:(tt + 1) * HD], pv[:])

        # ---------------- Phase 2: attention ----------------
        with ExitStack() as ctx:
            ppool = ctx.enter_context(tc.tile_pool(name="ppool", bufs=3))
            apool = ctx.enter_context(tc.tile_pool(name="apool", bufs=4))
            ptpool = ctx.enter_context(tc.tile_pool(name="ptpool", bufs=2))
            dpool = ctx.enter_context(tc.tile_pool(name="dpool", bufs=6))
            opool = ctx.enter_context(tc.tile_pool(name="opool", bufs=3))
            ps_s = ctx.enter_context(tc.tile_pool(name="ps_s", bufs=2, space="PSUM"))
            ps_t = ctx.enter_context(tc.tile_pool(name="ps_t", bufs=2, space="PSUM"))
            ps_o = ctx.enter_context(tc.tile_pool(name="ps_o", bufs=1, space="PSUM"))
            ps_f = ctx.enter_context(tc.tile_pool(name="ps_f", bufs=1, space="PSUM"))

            EXP_B = min(1024, S)          # exp/accum batch (2 psum banks)
            NEB = S // EXP_B
            EB = min(512, D)
            for sb in range(NSB):
                for h in range(HL):
                    pr, hp = h // 2, h % 2
                    lq0 = pr * S
                    pt_t = ptpool.tile([P, NTC * SB], F32R, tag="pt")
                    for st2 in range(SB // P):
                        st = sb * (SB // P) + st2
                        lq = qT[hp * DH:(hp + 1) * DH, lq0 + st * P: lq0 + (st + 1) * P]
                        p_sb = ppool.tile([P, S], F32, tag="p_sb")
                        dpart = dpool.tile([P, NEB + 2], F32, tag="dpart")
                        for eb in range(NEB):
                            ps = ps_s.tile([P, EXP_B], F32, tag="ps")
                            for q2 in range(EXP_B // TB):
                                t0 = eb * EXP_B + q2 * TB
                                nc.tensor.matmul(
                                    ps[:, q2 * TB:(q2 + 1) * TB], lq,
                                    kT[hp * DH:(hp + 1) * DH, lq0 + t0: lq0 + t0 + TB],
                                    start=True, stop=True)
                            nc.scalar.activation(
                                p_sb[:, eb * EXP_B:(eb + 1) * EXP_B], ps[:], AF.Exp,
                                scale=0.125, accum_out=dpart[:, eb:eb + 1])
                        # denom -> reciprocal
                        nc.vector.tensor_reduce(
                            dpart[:, NEB:NEB + 1], dpart[:, 0:NEB], axis=AX.X, op=ALU.add)
                        nc.vector.reciprocal(
                            dpart[:, NEB + 1:NEB + 2], dpart[:, NEB:NEB + 1])
                        rec = dpart[:, NEB + 1:NEB + 2]
                        # normalize (f32r) + store attn
                        a_sb = apool.tile([P, S], F32R, tag="a_sb")
                        for eb in range(NEB):
                            nc.vector.tensor_scalar_mul(
                                a_sb[:, eb * EXP_B:(eb + 1) * EXP_B],
                                p_sb[:, eb * EXP_B:(eb + 1) * EXP_B], rec)
                        nc.sync.dma_start(
                            attn_d[h, st * P:(st + 1) * P, :], a_sb[:].bitcast(F32))
                        # transpose attn tile -> PT strip
                        for tcg in range(NTC // 4):
                            pt_ps = ps_t.tile([P, 512], F32R, tag="pt_ps")
                            for j in range(4):
                                tc_i = tcg * 4 + j
                                nc.tensor.transpose(
                                    pt_ps[:, j * P:(j + 1) * P],
                                    a_sb[:, tc_i * P:(tc_i + 1) * P], ident_r[:])
                            dst = pt_t[:].rearrange("p (tc sb) -> p tc sb", sb=SB)[
                                :, tcg * 4:(tcg + 1) * 4, st2 * P:(st2 + 1) * P]
                            if (st2 * 4 + tcg) % 4 == 0:
                                nc.scalar.copy(dst, pt_ps[:].rearrange(
                                    "p (j q) -> p j q", j=4))
                            else:
                                nc.vector.tensor_copy(dst, pt_ps[:].rearrange(
                                    "p (j q) -> p j q", j=4))
                    # AV: OT[dh, s-block] += V_chunk.T @ PT_chunk
                    po = ps_o.tile([DH, SB], F32, tag="po")
                    for tc_i in range(NTC):
                        nc.tensor.matmul(
                            po[:],
                            V4[:, tc_i * HD + h * DH: tc_i * HD + (h + 1) * DH],
                            pt_t[:, tc_i * SB:(tc_i + 1) * SB],
                            start=(tc_i == 0), stop=(tc_i == NTC - 1))
                    nc.vector.tensor_copy(
                        OT2[hp * DH:(hp + 1) * DH, lq0 + sb * SB: lq0 + (sb + 1) * SB],
                        po[:])

                # output projection for this s-block once all heads done
                for st2 in range(SB // P):
                    st = sb * (SB // P) + st2
                    o_sb = opool.tile([P, D], F32, tag="o_sb")
                    for eb in range(D // EB):
                        pf = ps_f.tile([P, EB], F32, tag="pf")
                        for pr2 in range(PAIRS):
                            nc.tensor.matmul(
                                pf[:],
                                OT2[:, pr2 * S + st * P: pr2 * S + (st + 1) * P],
                                wo_r[:, pr2 * D + eb * EB: pr2 * D + (eb + 1) * EB],
                                start=(pr2 == 0), stop=(pr2 == PAIRS - 1))
                        nc.scalar.copy(o_sb[:, eb * EB:(eb + 1) * EB], pf[:])
                    nc.sync.dma_start(out_d[st * P:(st + 1) * P, :], o_sb[:])


def build(S=2048, D=1024, HL=4):
    nc = bacc.Bacc("TRN2", target_bir_lowering=False, debug=False,
                   enable_asserts=False, num_devices=8)
    HD = HL * DH
    ins = {
        "xq": nc.dram_tensor("xq", [S, D], F32, kind="ExternalInput").ap(),
        "xk": nc.dram_tensor("xk", [S, D], F32, kind="ExternalInput").ap(),
        "xv": nc.dram_tensor("xv", [S, D], F32, kind="ExternalInput").ap(),
        "wq": nc.dram_tensor("wq", [D, HD], F32, kind="ExternalInput").ap(),
        "wk": nc.dram_tensor("wk", [D, HD], F32, kind="ExternalInput").ap(),
        "wv": nc.dram_tensor("wv", [D, HD], F32, kind="ExternalInput").ap(),
        "wo": nc.dram_tensor("wo", [HD, D], F32, kind="ExternalInput").ap(),
    }
    outs = {
        "attn": nc.dram_tensor("attn", [HL, S, S], F32, kind="ExternalOutput").ap(),
        "out": nc.dram_tensor("out", [S, D], F32, kind="ExternalOutput").ap(),
    }
    with tile.TileContext(nc) as tc:
        emit(tc, outs, ins, S, D, HL)
    nc.compile()
    return nc


_NC_CACHE = {}


def _get_nc():
    if "nc" not in _NC_CACHE:
        _NC_CACHE["nc"] = build()
    return _NC_CACHE["nc"]


def kernel(query, key, value, Wq, Wk, Wv, Wo, _trace=False):
    B, S, D = 2, 2048, 1024
    H, HL = 16, 4
    query = np.ascontiguousarray(np.asarray(query, dtype=np.float32))
    key = np.ascontiguousarray(np.asarray(key, dtype=np.float32))
    value = np.ascontiguousarray(np.asarray(value, dtype=np.float32))
    Wq = np.asarray(Wq, dtype=np.float32)
    Wk = np.asarray(Wk, dtype=np.float32)
    Wv = np.asarray(Wv, dtype=np.float32)
    Wo = np.ascontiguousarray(np.asarray(Wo, dtype=np.float32))

    def warr(W, g):
        # [HL, D, DH] -> [D, HL*DH]
        return np.ascontiguousarray(
            W[g * HL:(g + 1) * HL].transpose(1, 0, 2).reshape(D, HL * DH))

    in_maps = []
    for c in range(8):
        b, g = c // 4, c % 4
        in_maps.append({
            "xq": query[b], "xk": key[b], "xv": value[b],
            "wq": warr(Wq, g), "wk": warr(Wk, g), "wv": warr(Wv, g),
            "wo": np.ascontiguousarray(Wo[g * HL * DH:(g + 1) * HL * DH]),
        })

    nc = _get_nc()
    res = run_bass_kernel_spmd(nc, in_maps, core_ids=list(range(8)), trace=_trace)

    attn = np.empty((B, H, S, S), dtype=np.float32)
    out = np.zeros((B, S, D), dtype=np.float32)
    for c in range(8):
        b, g = c // 4, c % 4
        attn[b, g * HL:(g + 1) * HL] = res.results[c]["attn"]
        out[b] += res.results[c]["out"]
    if _trace:
        return (out, attn), res
    return out, attn


# revision 16
# speedup vs baseline: 1.1092x; 1.1092x over previous
"""Multi-head attention Trainium2 Bass kernel.

Full problem: B=2, S=2048, HIDDEN=1024, 16 heads x d_head 64.
Sharding over 8 cores: batch (2) x head-group (4 groups of 4 heads).
Each core computes, for its batch b and heads [4g, 4g+4):
  - attn_part [4, S, S]  (normalized attention probabilities)
  - out_part  [S, HIDDEN] = concat(local heads) @ Wo[rows of local heads]
Host gathers: attn[b, 4g:4g+4] = attn_part;  out[b] = sum_g out_part.
"""

import numpy as np
from contextlib import ExitStack

import concourse.bass as bass
import concourse.tile as tile
from concourse import bacc, mybir
from concourse.bass_utils import run_bass_kernel_spmd
from concourse.masks import make_identity

P = 128
DH = 64                      # head dim
F32 = mybir.dt.float32
F32R = mybir.dt.float32r
AF = mybir.ActivationFunctionType
AX = mybir.AxisListType
ALU = mybir.AluOpType


def emit(tc, outs, ins, S, D, HL):
    """Emit one core's program.

    ins:  xq, xk, xv [S, D];  wq, wk, wv [D, HL*DH] (w[d, h*DH+k] = W[h,d,k]);
          wo [HL*DH, D]
    outs: attn [HL, S, S];  out [S, D]
    """
    nc = tc.nc
    assert S % 512 == 0 and D % P == 0 and HL % 2 == 0
    NCH = D // P            # d-chunks
    PAIRS = HL // 2         # head pairs (2 heads stacked on 128 partitions)
    NST = S // P            # s-tiles
    TB = min(512, S)        # t-block for scores/exp/normalize
    NTB = S // TB
    SB = min(256, S)        # s-block for PT strips / AV
    NSB = S // SB
    NTC = S // P            # t-chunks
    HD = HL * DH            # local concat width

    xq_d, xk_d, xv_d = ins["xq"], ins["xk"], ins["xv"]
    wq_d, wk_d, wv_d, wo_d = ins["wq"], ins["wk"], ins["wv"], ins["wo"]
    attn_d, out_d = outs["attn"], outs["out"]

    with ExitStack() as octx:
        const = octx.enter_context(tc.tile_pool(name="const", bufs=1))
        persist = octx.enter_context(tc.tile_pool(name="persist", bufs=1))

        ident_f = const.tile([P, P], F32)
        make_identity(nc, ident_f)
        ident_r = const.tile([P, P], F32R)
        nc.scalar.copy(ident_r[:], ident_f[:])

        # output-projection weights, rounded to f32r: [128, PAIRS, D]
        wo_f = const.tile([P, PAIRS * D], F32)
        nc.sync.dma_start(wo_f[:].rearrange("p (c n) -> p c n", c=PAIRS),
                          wo_d.rearrange("(c p) n -> p c n", p=P))
        wo_r = const.tile([P, PAIRS * D], F32R)
        nc.scalar.copy(wo_r[:], wo_f[:])

        # persistent activations (f32r; feed fp32r matmuls)
        qT = persist.tile([P, PAIRS * S], F32R)   # [2x64 dh, pair-major s]
        kT = persist.tile([P, PAIRS * S], F32R)
        V4 = persist.tile([P, NST * HD], F32R)    # [t, t-tile-major (h, k)]
        OT2 = persist.tile([P, PAIRS * S], F32R)  # [2x64 dh, pair-major s]

        # ---------------- Phase 1: transposes + projections ----------------
        with ExitStack() as ctx:
            xpool = ctx.enter_context(tc.tile_pool(name="xpool", bufs=6))
            xtpool = ctx.enter_context(tc.tile_pool(name="xtpool", bufs=2))
            wfpool = ctx.enter_context(tc.tile_pool(name="wfpool", bufs=2))
            wrpool = ctx.enter_context(tc.tile_pool(name="wrpool", bufs=3))
            ps_xt = ctx.enter_context(tc.tile_pool(name="ps_xt", bufs=3, space="PSUM"))
            ps_pj = ctx.enter_context(tc.tile_pool(name="ps_pj", bufs=3, space="PSUM"))

            for x_d, w_d, kind in ((xk_d, wk_d, "k"), (xq_d, wq_d, "q"),
                                   (xv_d, wv_d, "v")):
                # round projection weights to f32r: [128, NCH, HL*DH]
                w_f = wfpool.tile([P, NCH * HD], F32, tag="w_f")
                nc.sync.dma_start(w_f[:].rearrange("p (c n) -> p c n", c=NCH),
                                  w_d.rearrange("(c p) n -> p c n", p=P))
                w_r = wrpool.tile([P, NCH * HD], F32R, tag="w_r")
                nc.scalar.copy(w_r[:], w_f[:])

                for sblk in range(S // 512):
                    # x natural tiles for this 512-row slab
                    xts = []
                    for st2 in range(4):
                        x_t = xpool.tile([P, D], F32, tag="x_t")
                        nc.sync.dma_start(
                            x_t[:], x_d[sblk * 512 + st2 * P: sblk * 512 + (st2 + 1) * P, :])
                        x_r = xpool.tile([P, D], F32R, tag="x_r")
                        nc.vector.tensor_copy(x_r[:], x_t[:])
                        xts.append(x_r)
                    # transpose slab -> xT [128, NCH*512] (f32r)
                    xT = xtpool.tile([P, NCH * 512], F32R, tag="xT")
                    for c in range(NCH):
                        pxt = ps_xt.tile([P, 512], F32R, tag="pxt")
                        for st2 in range(4):
                            nc.tensor.transpose(
                                pxt[:, st2 * P:(st2 + 1) * P],
                                xts[st2][:, c * P:(c + 1) * P], ident_r[:])
                        nc.scalar.copy(xT[:, c * 512:(c + 1) * 512], pxt[:])

                    if kind in ("q", "k"):
                        dst = qT if kind == "q" else kT
                        for pr in range(PAIRS):
                            pp = ps_pj.tile([P, 512], F32, tag="pp")
                            for c in range(NCH):
                                nc.tensor.matmul(
                                    pp[:],
                                    w_r[:, c * HD + pr * P: c * HD + (pr + 1) * P],
                                    xT[:, c * 512:(c + 1) * 512],
                                    start=(c == 0), stop=(c == NCH - 1))
                            nc.scalar.copy(
                                dst[:, pr * S + sblk * 512: pr * S + (sblk + 1) * 512],
                                pp[:])
                    else:
                        for tt2 in range(4):
                            tt = sblk * 4 + tt2
                            pv = ps_pj.tile([P, HD], F32, tag="pp")
                            for c in range(NCH):
                                nc.tensor.matmul(
                                    pv[:],
                                    xT[:, c * 512 + tt2 * P: c * 512 + (tt2 + 1) * P],
                                    w_r[:, c * HD:(c + 1) * HD],
                                    start=(c == 0), stop=(c == NCH - 1))
                            nc.vector.tensor_copy(
                                V4[:, tt * HD:(tt + 1) * HD], pv[:])

        # ---------------- Phase 2: attention ----------------
        with ExitStack() as ctx:
            ppool = ctx.enter_context(tc.tile_pool(name="ppool", bufs=3))
            apool = ctx.enter_context(tc.tile_pool(name="apool", bufs=4))
            ptpool = ctx.enter_context(tc.tile_pool(name="ptpool", bufs=3))
            dpool = ctx.enter_context(tc.tile_pool(name="dpool", bufs=6))
            opool = ctx.enter_context(tc.tile_pool(name="opool", bufs=3))
            ps_s = ctx.enter_context(tc.tile_pool(name="ps_s", bufs=2, space="PSUM"))
            ps_t = ctx.enter_context(tc.tile_pool(name="ps_t", bufs=2, space="PSUM"))
            ps_o = ctx.enter_context(tc.tile_pool(name="ps_o", bufs=1, space="PSUM"))
            ps_f = ctx.enter_context(tc.tile_pool(name="ps_f", bufs=1, space="PSUM"))

            EXP_B = min(1024, S)          # exp/accum batch (2 psum banks)
            NEB = S // EXP_B
            EB = min(512, D)
            for sb in range(NSB):
                for h in range(HL):
                    pr, hp = h // 2, h % 2
                    lq0 = pr * S
                    pt_t = ptpool.tile([P, NTC * SB], F32R, tag="pt")
                    for st2 in range(SB // P):
                        st = sb * (SB // P) + st2
                        lq = qT[hp * DH:(hp + 1) * DH, lq0 + st * P: lq0 + (st + 1) * P]
                        p_sb = ppool.tile([P, S], F32, tag="p_sb")
                        dpart = dpool.tile([P, NEB + 2], F32, tag="dpart")
                        for eb in range(NEB):
                            ps = ps_s.tile([P, EXP_B], F32, tag="ps")
                            for q2 in range(EXP_B // TB):
                                t0 = eb * EXP_B + q2 * TB
                                nc.tensor.matmul(
                                    ps[:, q2 * TB:(q2 + 1) * TB], lq,
                                    kT[hp * DH:(hp + 1) * DH, lq0 + t0: lq0 + t0 + TB],
                                    start=True, stop=True)
                            nc.scalar.activation(
                                p_sb[:, eb * EXP_B:(eb + 1) * EXP_B], ps[:], AF.Exp,
                                scale=0.125, accum_out=dpart[:, eb:eb + 1])
                        # denom -> reciprocal
                        nc.vector.tensor_reduce(
                            dpart[:, NEB:NEB + 1], dpart[:, 0:NEB], axis=AX.X, op=ALU.add)
                        nc.vector.reciprocal(
                            dpart[:, NEB + 1:NEB + 2], dpart[:, NEB:NEB + 1])
                        rec = dpart[:, NEB + 1:NEB + 2]
                        # normalize (f32r) + store attn
                        a_sb = apool.tile([P, S], F32R, tag="a_sb")
                        for eb in range(NEB):
                            nc.vector.tensor_scalar_mul(
                                a_sb[:, eb * EXP_B:(eb + 1) * EXP_B],
                                p_sb[:, eb * EXP_B:(eb + 1) * EXP_B], rec)
                        nc.sync.dma_start(
                            attn_d[h, st * P:(st + 1) * P, :], a_sb[:].bitcast(F32))
                        # transpose attn tile -> PT strip
                        for tcg in range(NTC // 4):
                            pt_ps = ps_t.tile([P, 512], F32R, tag="pt_ps")
                            for j in range(4):
                                tc_i = tcg * 4 + j
                                nc.tensor.transpose(
                                    pt_ps[:, j * P:(j + 1) * P],
                                    a_sb[:, tc_i * P:(tc_i + 1) * P], ident_r[:])
                            dst = pt_t[:].rearrange("p (tc sb) -> p tc sb", sb=SB)[
                                :, tcg * 4:(tcg + 1) * 4, st2 * P:(st2 + 1) * P]
                            if (st2 * 4 + tcg) % 4 == 0:
                                nc.scalar.copy(dst, pt_ps[:].rearrange(
                                    "p (j q) -> p j q", j=4))
                            else:
                                nc.vector.tensor_copy(dst, pt_ps[:].rearrange(
                                    "p (j q) -> p j q", j=4))
                    # AV: OT[dh, s-block] += V_chunk.T @ PT_chunk
                    po = ps_o.tile([DH, SB], F32, tag="po")
                    for tc_i in range(NTC):
                        nc.tensor.matmul(
                            po[:],
                            V4[:, tc_i * HD + h * DH: tc_i * HD + (h + 1) * DH],
                            pt_t[:, tc_i * SB:(tc_i + 1) * SB],
                            start=(tc_i == 0), stop=(tc_i == NTC - 1))
                    nc.vector.tensor_copy(
                        OT2[hp * DH:(hp + 1) * DH, lq0 + sb * SB: lq0 + (sb + 1) * SB],
                        po[:])

                # output projection for this s-block once all heads done
                for st2 in range(SB // P):
                    st = sb * (SB // P) + st2
                    o_sb = opool.tile([P, D], F32, tag="o_sb")
                    for eb in range(D // EB):
                        pf = ps_f.tile([P, EB], F32, tag="pf")
                        for pr2 in range(PAIRS):
                            nc.tensor.matmul(
                                pf[:],
                                OT2[:, pr2 * S + st * P: pr2 * S + (st + 1) * P],
                                wo_r[:, pr2 * D + eb * EB: pr2 * D + (eb + 1) * EB],
                                start=(pr2 == 0), stop=(pr2 == PAIRS - 1))
                        nc.scalar.copy(o_sb[:, eb * EB:(eb + 1) * EB], pf[:])
                    nc.sync.dma_start(out_d[st * P:(st + 1) * P, :], o_sb[:])


def build(S=2048, D=1024, HL=4):
    nc = bacc.Bacc("TRN2", target_bir_lowering=False, debug=False,
                   enable_asserts=False, num_devices=8)
    HD = HL * DH
    ins = {
        "xq": nc.dram_tensor("xq", [S, D], F32, kind="ExternalInput").ap(),
        "xk": nc.dram_tensor("xk", [S, D], F32, kind="ExternalInput").ap(),
        "xv": nc.dram_tensor("xv", [S, D], F32, kind="ExternalInput").ap(),
        "wq": nc.dram_tensor("wq", [D, HD], F32, kind="ExternalInput").ap(),
        "wk": nc.dram_tensor("wk", [D, HD], F32, kind="ExternalInput").ap(),
        "wv": nc.dram_tensor("wv", [D, HD], F32, kind="ExternalInput").ap(),
        "wo": nc.dram_tensor("wo", [HD, D], F32, kind="ExternalInput").ap(),
    }
    outs = {
        "attn": nc.dram_tensor("attn", [HL, S, S], F32, kind="ExternalOutput").ap(),
        "out": nc.dram_tensor("out", [S, D], F32, kind="ExternalOutput").ap(),
    }
    with tile.TileContext(nc) as tc:
        emit(tc, outs, ins, S, D, HL)
    nc.compile()
    return nc


_NC_CACHE = {}


def _get_nc():
    if "nc" not in _NC_CACHE:
        _NC_CACHE["nc"] = build()
    return _NC_CACHE["nc"]


def kernel(query, key, value, Wq, Wk, Wv, Wo, _trace=False):
    B, S, D = 2, 2048, 1024
    H, HL = 16, 4
    query = np.ascontiguousarray(np.asarray(query, dtype=np.float32))
    key = np.ascontiguousarray(np.asarray(key, dtype=np.float32))
    value = np.ascontiguousarray(np.asarray(value, dtype=np.float32))
    Wq = np.asarray(Wq, dtype=np.float32)
    Wk = np.asarray(Wk, dtype=np.float32)
    Wv = np.asarray(Wv, dtype=np.float32)
    Wo = np.ascontiguousarray(np.asarray(Wo, dtype=np.float32))

    def warr(W, g):
        # [HL, D, DH] -> [D, HL*DH]
        return np.ascontiguousarray(
            W[g * HL:(g + 1) * HL].transpose(1, 0, 2).reshape(D, HL * DH))

    in_maps = []
    for c in range(8):
        b, g = c // 4, c % 4
        in_maps.append({
            "xq": query[b], "xk": key[b], "xv": value[b],
            "wq": warr(Wq, g), "wk": warr(Wk, g), "wv": warr(Wv, g),
            "wo": np.ascontiguousarray(Wo[g * HL * DH:(g + 1) * HL * DH]),
        })

    nc = _get_nc()
    res = run_bass_kernel_spmd(nc, in_maps, core_ids=list(range(8)), trace=_trace)

    attn = np.empty((B, H, S, S), dtype=np.float32)
    out = np.zeros((B, S, D), dtype=np.float32)
    for c in range(8):
        b, g = c // 4, c % 4
        attn[b, g * HL:(g + 1) * HL] = res.results[c]["attn"]
        out[b] += res.results[c]["out"]
    if _trace:
        return (out, attn), res
    return out, attn


# revision 18
# speedup vs baseline: 1.1421x; 1.0297x over previous
"""Multi-head attention Trainium2 Bass kernel.

Full problem: B=2, S=2048, HIDDEN=1024, 16 heads x d_head 64.
Sharding over 8 cores: batch (2) x head-group (4 groups of 4 heads).
Each core computes, for its batch b and heads [4g, 4g+4):
  - attn_part [4, S, S]  (normalized attention probabilities)
  - out_part  [S, HIDDEN] = concat(local heads) @ Wo[rows of local heads]
Host gathers: attn[b, 4g:4g+4] = attn_part;  out[b] = sum_g out_part.
"""

import numpy as np
from contextlib import ExitStack

import concourse.bass as bass
import concourse.tile as tile
from concourse import bacc, mybir
from concourse.bass_utils import run_bass_kernel_spmd
from concourse.masks import make_identity

P = 128
DH = 64                      # head dim
F32 = mybir.dt.float32
F32R = mybir.dt.float32r
AF = mybir.ActivationFunctionType
AX = mybir.AxisListType
ALU = mybir.AluOpType


def emit(tc, outs, ins, S, D, HL):
    """Emit one core's program.

    ins:  xq, xk, xv [S, D];  wq, wk, wv [D, HL*DH] (w[d, h*DH+k] = W[h,d,k]);
          wo [HL*DH, D]
    outs: attn [HL, S, S];  out [S, D]
    """
    nc = tc.nc
    assert S % 512 == 0 and D % P == 0 and HL % 2 == 0
    NCH = D // P            # d-chunks
    PAIRS = HL // 2         # head pairs (2 heads stacked on 128 partitions)
    NST = S // P            # s-tiles
    TB = min(512, S)        # t-block for scores/exp/normalize
    NTB = S // TB
    SB = min(256, S)        # s-block for PT strips / AV
    NSB = S // SB
    NTC = S // P            # t-chunks
    HD = HL * DH            # local concat width

    xq_d, xk_d, xv_d = ins["xq"], ins["xk"], ins["xv"]
    wq_d, wk_d, wv_d, wo_d = ins["wq"], ins["wk"], ins["wv"], ins["wo"]
    attn_d, out_d = outs["attn"], outs["out"]

    with ExitStack() as octx:
        const = octx.enter_context(tc.tile_pool(name="const", bufs=1))
        persist = octx.enter_context(tc.tile_pool(name="persist", bufs=1))

        ident_f = const.tile([P, P], F32)
        make_identity(nc, ident_f)
        ident_r = const.tile([P, P], F32R)
        nc.scalar.copy(ident_r[:], ident_f[:])

        # output-projection weights, rounded to f32r: [128, PAIRS, D]
        wo_f = const.tile([P, PAIRS * D], F32)
        nc.sync.dma_start(wo_f[:].rearrange("p (c n) -> p c n", c=PAIRS),
                          wo_d.rearrange("(c p) n -> p c n", p=P))
        wo_r = const.tile([P, PAIRS * D], F32R)
        nc.scalar.copy(wo_r[:], wo_f[:])

        # persistent activations (f32r; feed fp32r matmuls)
        qT = persist.tile([P, PAIRS * S], F32R)   # [2x64 dh, pair-major s]
        kT = persist.tile([P, PAIRS * S], F32R)
        V4 = persist.tile([P, NST * HD], F32R)    # [t, t-tile-major (h, k)]
        OT2 = persist.tile([P, PAIRS * S], F32R)  # [2x64 dh, pair-major s]

        # ---------------- Phase 1: transposes + projections ----------------
        with ExitStack() as ctx:
            xpool = ctx.enter_context(tc.tile_pool(name="xpool", bufs=6))
            xtpool = ctx.enter_context(tc.tile_pool(name="xtpool", bufs=2))
            wfpool = ctx.enter_context(tc.tile_pool(name="wfpool", bufs=2))
            wrpool = ctx.enter_context(tc.tile_pool(name="wrpool", bufs=3))
            ps_xt = ctx.enter_context(tc.tile_pool(name="ps_xt", bufs=3, space="PSUM"))
            ps_pj = ctx.enter_context(tc.tile_pool(name="ps_pj", bufs=3, space="PSUM"))

            for x_d, w_d, kind in ((xk_d, wk_d, "k"), (xq_d, wq_d, "q"),
                                   (xv_d, wv_d, "v")):
                # round projection weights to f32r: [128, NCH, HL*DH]
                w_f = wfpool.tile([P, NCH * HD], F32, tag="w_f")
                nc.sync.dma_start(w_f[:].rearrange("p (c n) -> p c n", c=NCH),
                                  w_d.rearrange("(c p) n -> p c n", p=P))
                w_r = wrpool.tile([P, NCH * HD], F32R, tag="w_r")
                nc.scalar.copy(w_r[:], w_f[:])

                for sblk in range(S // 512):
                    # x natural tiles for this 512-row slab
                    xts = []
                    for st2 in range(4):
                        x_t = xpool.tile([P, D], F32, tag="x_t")
                        nc.sync.dma_start(
                            x_t[:], x_d[sblk * 512 + st2 * P: sblk * 512 + (st2 + 1) * P, :])
                        x_r = xpool.tile([P, D], F32R, tag="x_r")
                        nc.vector.tensor_copy(x_r[:], x_t[:])
                        xts.append(x_r)
                    # transpose slab -> xT [128, NCH*512] (f32r)
                    xT = xtpool.tile([P, NCH * 512], F32R, tag="xT")
                    for c in range(NCH):
                        pxt = ps_xt.tile([P, 512], F32R, tag="pxt")
                        for st2 in range(4):
                            nc.tensor.transpose(
                                pxt[:, st2 * P:(st2 + 1) * P],
                                xts[st2][:, c * P:(c + 1) * P], ident_r[:])
                        nc.scalar.copy(xT[:, c * 512:(c + 1) * 512], pxt[:])

                    if kind in ("q", "k"):
                        dst = qT if kind == "q" else kT
                        for pr in range(PAIRS):
                            pp = ps_pj.tile([P, 512], F32, tag="pp")
                            for c in range(NCH):
                                nc.tensor.matmul(
                                    pp[:],
                                    w_r[:, c * HD + pr * P: c * HD + (pr + 1) * P],
                                    xT[:, c * 512:(c + 1) * 512],
                                    start=(c == 0), stop=(c == NCH - 1))
                            nc.scalar.copy(
                                dst[:, pr * S + sblk * 512: pr * S + (sblk + 1) * 512],
                                pp[:])
                    else:
                        for tt2 in range(4):
                            tt = sblk * 4 + tt2
                            pv = ps_pj.tile([P, HD], F32, tag="pp")
                            for c in range(NCH):
                                nc.tensor.matmul(
                                    pv[:],
                                    xT[:, c * 512 + tt2 * P: c * 512 + (tt2 + 1) * P],
                                    w_r[:, c * HD:(c + 1) * HD],
                                    start=(c == 0), stop=(c == NCH - 1))
                            nc.vector.tensor_copy(
                                V4[:, tt * HD:(tt + 1) * HD], pv[:])

        # ---------------- Phase 2: attention ----------------
        with ExitStack() as ctx:
            ppool = ctx.enter_context(tc.tile_pool(name="ppool", bufs=3))
            apool = ctx.enter_context(tc.tile_pool(name="apool", bufs=4))
            ptpool = ctx.enter_context(tc.tile_pool(name="ptpool", bufs=3))
            dpool = ctx.enter_context(tc.tile_pool(name="dpool", bufs=6))
            opool = ctx.enter_context(tc.tile_pool(name="opool", bufs=3))
            ps_s = ctx.enter_context(tc.tile_pool(name="ps_s", bufs=2, space="PSUM"))
            ps_t = ctx.enter_context(tc.tile_pool(name="ps_t", bufs=2, space="PSUM"))
            ps_o = ctx.enter_context(tc.tile_pool(name="ps_o", bufs=1, space="PSUM"))
            ps_f = ctx.enter_context(tc.tile_pool(name="ps_f", bufs=1, space="PSUM"))

            EXP_B = min(1024, S)          # exp/accum batch (2 psum banks)
            NEB = S // EXP_B
            EB = min(512, D)
            for sb in range(NSB):
                for h in range(HL):
                    pr, hp = h // 2, h % 2
                    lq0 = pr * S
                    pt_t = ptpool.tile([P, NTC * SB], F32R, tag="pt")
                    for st2 in range(SB // P):
                        st = sb * (SB // P) + st2
                        lq = qT[hp * DH:(hp + 1) * DH, lq0 + st * P: lq0 + (st + 1) * P]
                        p_sb = ppool.tile([P, S], F32, tag="p_sb")
                        dpart = dpool.tile([P, NEB + 2], F32, tag="dpart")
                        for eb in range(NEB):
                            ps = ps_s.tile([P, EXP_B], F32, tag="ps")
                            for q2 in range(EXP_B // TB):
                                t0 = eb * EXP_B + q2 * TB
                                nc.tensor.matmul(
                                    ps[:, q2 * TB:(q2 + 1) * TB], lq,
                                    kT[hp * DH:(hp + 1) * DH, lq0 + t0: lq0 + t0 + TB],
                                    start=True, stop=True)
                            nc.scalar.activation(
                                p_sb[:, eb * EXP_B:(eb + 1) * EXP_B], ps[:], AF.Exp,
                                scale=0.125, accum_out=dpart[:, eb:eb + 1])
                        # denom -> reciprocal
                        nc.vector.tensor_reduce(
                            dpart[:, NEB:NEB + 1], dpart[:, 0:NEB], axis=AX.X, op=ALU.add)
                        nc.vector.reciprocal(
                            dpart[:, NEB + 1:NEB + 2], dpart[:, NEB:NEB + 1])
                        rec = dpart[:, NEB + 1:NEB + 2]
                        # normalize (f32r) + store attn
                        a_sb = apool.tile([P, S], F32R, tag="a_sb")
                        for eb in range(NEB):
                            nc.vector.tensor_scalar_mul(
                                a_sb[:, eb * EXP_B:(eb + 1) * EXP_B],
                                p_sb[:, eb * EXP_B:(eb + 1) * EXP_B], rec)
                        nc.sync.dma_start(
                            attn_d[h, st * P:(st + 1) * P, :], a_sb[:].bitcast(F32))
                        # transpose attn tile -> PT strip
                        for tcg in range(NTC // 4):
                            pt_ps = ps_t.tile([P, 512], F32R, tag="pt_ps")
                            for j in range(4):
                                tc_i = tcg * 4 + j
                                nc.tensor.transpose(
                                    pt_ps[:, j * P:(j + 1) * P],
                                    a_sb[:, tc_i * P:(tc_i + 1) * P], ident_r[:])
                            dst = pt_t[:].rearrange("p (tc sb) -> p tc sb", sb=SB)[
                                :, tcg * 4:(tcg + 1) * 4, st2 * P:(st2 + 1) * P]
                            if (st2 * 4 + tcg) % 4 == 0:
                                nc.scalar.copy(dst, pt_ps[:].rearrange(
                                    "p (j q) -> p j q", j=4))
                            else:
                                nc.vector.tensor_copy(dst, pt_ps[:].rearrange(
                                    "p (j q) -> p j q", j=4))
                    # AV: OT[dh, s-block] += V_chunk.T @ PT_chunk
                    po = ps_o.tile([DH, SB], F32, tag="po")
                    for tc_i in range(NTC):
                        nc.tensor.matmul(
                            po[:],
                            V4[:, tc_i * HD + h * DH: tc_i * HD + (h + 1) * DH],
                            pt_t[:, tc_i * SB:(tc_i + 1) * SB],
                            start=(tc_i == 0), stop=(tc_i == NTC - 1))
                    nc.vector.tensor_copy(
                        OT2[hp * DH:(hp + 1) * DH, lq0 + sb * SB: lq0 + (sb + 1) * SB],
                        po[:])

                # output projection for this s-block once all heads done
                for st2 in range(SB // P):
                    st = sb * (SB // P) + st2
                    o_sb = opool.tile([P, D], F32, tag="o_sb")
                    for eb in range(D // EB):
                        pf = ps_f.tile([P, EB], F32, tag="pf")
                        for pr2 in range(PAIRS):
                            nc.tensor.matmul(
                                pf[:],
                                OT2[:, pr2 * S + st * P: pr2 * S + (st + 1) * P],
                                wo_r[:, pr2 * D + eb * EB: pr2 * D + (eb + 1) * EB],
                                start=(pr2 == 0), stop=(pr2 == PAIRS - 1))
                        nc.scalar.copy(o_sb[:, eb * EB:(eb + 1) * EB], pf[:])
                    nc.sync.dma_start(out_d[st * P:(st + 1) * P, :], o_sb[:])


def build(S=2048, D=1024, HL=4):
    nc = bacc.Bacc("TRN2", target_bir_lowering=False, debug=False,
                   enable_asserts=False, num_devices=8)
    HD = HL * DH
    ins = {
        "xq": nc.dram_tensor("xq", [S, D], F32, kind="ExternalInput").ap(),
        "xk": nc.dram_tensor("xk", [S, D], F32, kind="ExternalInput").ap(),
        "xv": nc.dram_tensor("xv", [S, D], F32, kind="ExternalInput").ap(),
        "wq": nc.dram_tensor("wq", [D, HD], F32, kind="ExternalInput").ap(),
        "wk": nc.dram_tensor("wk", [D, HD], F32, kind="ExternalInput").ap(),
        "wv": nc.dram_tensor("wv", [D, HD], F32, kind="ExternalInput").ap(),
        "wo": nc.dram_tensor("wo", [HD, D], F32, kind="ExternalInput").ap(),
    }
    outs = {
        "attn": nc.dram_tensor("attn", [HL, S, S], F32, kind="ExternalOutput").ap(),
        "out": nc.dram_tensor("out", [S, D], F32, kind="ExternalOutput").ap(),
    }
    with tile.TileContext(nc) as tc:
        emit(tc, outs, ins, S, D, HL)
    nc.compile()
    return nc


_NC_CACHE = {}


def _get_nc():
    if "nc" not in _NC_CACHE:
        _NC_CACHE["nc"] = build()
    return _NC_CACHE["nc"]


def kernel(query, key, value, Wq, Wk, Wv, Wo):
    B, S, D = 2, 2048, 1024
    H, HL = 16, 4
    query = np.ascontiguousarray(np.asarray(query, dtype=np.float32))
    key = np.ascontiguousarray(np.asarray(key, dtype=np.float32))
    value = np.ascontiguousarray(np.asarray(value, dtype=np.float32))
    Wq = np.asarray(Wq, dtype=np.float32)
    Wk = np.asarray(Wk, dtype=np.float32)
    Wv = np.asarray(Wv, dtype=np.float32)
    Wo = np.ascontiguousarray(np.asarray(Wo, dtype=np.float32))

    def warr(W, g):
        # [HL, D, DH] -> [D, HL*DH]
        return np.ascontiguousarray(
            W[g * HL:(g + 1) * HL].transpose(1, 0, 2).reshape(D, HL * DH))

    in_maps = []
    for c in range(8):
        b, g = c // 4, c % 4
        in_maps.append({
            "xq": query[b], "xk": key[b], "xv": value[b],
            "wq": warr(Wq, g), "wk": warr(Wk, g), "wv": warr(Wv, g),
            "wo": np.ascontiguousarray(Wo[g * HL * DH:(g + 1) * HL * DH]),
        })

    nc = _get_nc()
    res = run_bass_kernel_spmd(nc, in_maps, core_ids=list(range(8)))

    attn = np.empty((B, H, S, S), dtype=np.float32)
    out = np.zeros((B, S, D), dtype=np.float32)
    for c in range(8):
        b, g = c // 4, c % 4
        attn[b, g * HL:(g + 1) * HL] = res.results[c]["attn"]
        out[b] += res.results[c]["out"]
    return out, attn
